# revision 31
# baseline (speedup 1.0000x reference)
"""Trainium2 Bass kernel for a 4-head spatial MultiHeadAttention block.

Reference computation (per batch n):
    q/k/v = 1x1-conv projections of x (C=256 channels, S=48*48=2304 positions)
    per head (4 heads, d=64): attn = softmax(q^T k / 8), out = attn @ v
    out = Wo @ concat(heads) + bo + x   (residual)

Sharding across 8 NeuronCores: core c handles batch n = c//2 and head-pair
hp = c%2 (output channels [hp*128, hp*128+128) of the QKV projections, i.e.
heads {2*hp, 2*hp+1}).  Each core computes a partial output
Wo[:, ch] @ attn_ch (256 x 2304); the host sums the two partials per batch
and adds bo + residual x.

The ScalarE (ACT) exp of the 2*2304^2 scores is the hard floor (~83us at
128 lanes / 1.2 GHz); the schedule is built so ACT never waits:
  - startup: only K/Q of s-chunk 0 are computed up front (first exp at ~3us);
    the remaining K chunks + all V tiles weave into unit 0's groups, Q chunks
    into units 1-2, Wo chunks into later units' spare slots.
  - tail: the last chunk's Wo uses a split 64-row contraction reading the
    head-B normalized tile directly, skipping the partition-shift DMA.
  - normalization is DMA-free (each DMA hop costs ~3us of completion
    latency): reciprocal_approx_fast on the (1, sw) sums row, a bf16 cast,
    then a partition-broadcast matmul against a ones-row stationary (same
    128x128 PE config as every other matmul) into the spare "ot" PSUM slot.
Per-core kernel layout (as in the earlier revision):
  - Q stored (d, s) with both heads stacked on partitions; K zero-padded per
    head (kz0/kz1) so every scores matmul contracts 128 partitions (no PE
    tile-config switches); V produced transposed with a ones column so attn@V
    also yields softmax row-sums (M=65).
  - scoresT(t,s): 3 t-tiles per 3-bank PSUM tile; exp (ScalarE) runs 1536-wide
    straight out of PSUM; attn@V of batch g is emitted after the scores of
    batch g+1 so the PE never waits on ScalarE.
  - normalization: reciprocal on a (64, sw/64) lane-spread reshape, then
    partition-broadcast via an SBUF bounce on the gpsimd queue.
All matmul operands are bf16; accumulation and softmax math are fp32.
"""

from collections import deque

import numpy as np

import concourse.bass as bass
import concourse.mybir as mybir
import concourse.tile as tile
from concourse import bacc
from concourse.bass_utils import run_bass_kernel_spmd

C = 256          # channels
S = 2304         # spatial positions (48*48)
HD = 64          # head dim
P = 128          # partitions
TT = S // P      # 18 t-tiles of 128
GRP = 3          # t-tiles per exp batch (3 PSUM banks)
SCALE = 0.125    # 1/sqrt(HD)
F32 = mybir.dt.float32
BF16 = mybir.dt.bfloat16

S_CHUNKS = [(0, 512), (512, 512), (1024, 512), (1536, 512), (2048, 256)]


def _body(tc):
    nc = tc.nc
    t_x = nc.dram_tensor("x", [C, S], BF16, kind="ExternalInput").ap()
    t_wqt = nc.dram_tensor("wqt", [C, P], BF16, kind="ExternalInput").ap()
    t_wkt = nc.dram_tensor("wkt", [C, P], BF16, kind="ExternalInput").ap()
    t_wvt = nc.dram_tensor("wvt", [C, P], BF16, kind="ExternalInput").ap()
    t_wot = nc.dram_tensor("wot", [P, C], BF16, kind="ExternalInput").ap()
    t_bq = nc.dram_tensor("bq", [P, 1], F32, kind="ExternalInput").ap()
    t_bk = nc.dram_tensor("bk", [P, 1], F32, kind="ExternalInput").ap()
    t_bv = nc.dram_tensor("bv", [1, P], F32, kind="ExternalInput").ap()
    t_out = nc.dram_tensor("out", [C, S], F32, kind="ExternalOutput").ap()

    singles = tc.alloc_tile_pool(name="singles", bufs=1)
    x_lo = singles.tile([P, S], BF16)
    x_hi = singles.tile([P, S], BF16)
    q_sb = singles.tile([P, S], BF16)
    kz0 = singles.tile([P, S], BF16)          # head A rows 0-63, zeros 64-127
    kz1 = singles.tile([P, S], BF16)          # zeros 0-63, head B rows 64-127
    vt_sb = singles.tile([P, TT, 130], BF16)  # per tt: [dA(64) | 1 | dB(64) | 1]
    wq_sb = singles.tile([P, 2, P], BF16)
    wk_sb = singles.tile([P, 2, P], BF16)
    wv_sb = singles.tile([P, 2, P], BF16)
    wot_sb = singles.tile([P, C], BF16)
    wot_hi = singles.tile([HD, C], BF16)      # wot rows 64-127 at partitions 0-63
    attn_full = singles.tile([P, S], BF16)
    bq_sb = singles.tile([P, 1], F32)
    bk_sb = singles.tile([P, 1], F32)
    bv_bc = singles.tile([P, P], F32)
    ones64 = singles.tile([P, P], BF16)    # row 64 ones, rest zero: broadcaster
    # bf16 staging for the sums-broadcast matmul: row 64 gets the sums row
    # (same partition - DVE ops cannot cross partitions), the rest stays zero
    comb16 = singles.tile([P, 512], BF16)

    # Only the two ones-columns of vt need setting - the V projection adds
    # overwrite every other column.  The dead kz halves are zeroed per-chunk
    # inside k_chunk (a full-width memset would gate the first scores).
    nc.vector.memset(vt_sb[:, :, HD:HD + 1], 1.0)
    nc.vector.memset(vt_sb[:, :, 129:130], 1.0)
    nc.vector.memset(ones64, 0.0)
    nc.vector.memset(ones64[HD:HD + 1, :], 1.0)
    nc.vector.memset(comb16, 0.0)

    # ---- input DMAs: sync carries wk/wq + x chunks 0,2,4; gpsimd carries the
    # small biases + wv + x chunks 1,3 + the (late-needed) wot tiles ----
    nc.gpsimd.dma_start(out=wk_sb, in_=t_wkt.rearrange("(a p) d -> p a d", p=P))
    nc.sync.dma_start(out=x_lo[:, 0:512], in_=t_x[0:P, 0:512])
    nc.sync.dma_start(out=x_hi[:, 0:512], in_=t_x[P:C, 0:512])
    nc.gpsimd.dma_start(out=wq_sb, in_=t_wqt.rearrange("(a p) d -> p a d", p=P))
    nc.sync.dma_start(out=bk_sb, in_=t_bk)
    nc.sync.dma_start(out=bq_sb, in_=t_bq)
    nc.gpsimd.dma_start(out=wv_sb, in_=t_wvt.rearrange("(a p) d -> p a d", p=P))
    nc.gpsimd.dma_start(out=bv_bc, in_=t_bv.to_broadcast([P, P]))
    for ci, (s0, sw) in enumerate(S_CHUNKS[1:]):
        eng = nc.gpsimd if ci % 2 == 0 else nc.sync
        eng.dma_start(out=x_lo[:, s0:s0 + sw], in_=t_x[0:P, s0:s0 + sw])
        eng.dma_start(out=x_hi[:, s0:s0 + sw], in_=t_x[P:C, s0:s0 + sw])
    nc.gpsimd.dma_start(out=wot_sb, in_=t_wot)
    nc.gpsimd.dma_start(out=wot_hi, in_=t_wot[HD:P, :])

    ps = tc.alloc_tile_pool(name="ps", bufs=2, space="PSUM")
    ex_pool = tc.alloc_tile_pool(name="ex_sb", bufs=4)
    nrm = tc.alloc_tile_pool(name="nrm", bufs=2)
    wo_out = tc.alloc_tile_pool(name="wo_out", bufs=4)

    def k_chunk(s0, sw):
        psn = ps.tile([P, GRP * 512], F32, tag="sc", name="kps")[:, :sw]
        nc.tensor.matmul(psn, wk_sb[:, 0, :], x_lo[:, s0:s0 + sw],
                         start=True, stop=False)
        nc.tensor.matmul(psn, wk_sb[:, 1, :], x_hi[:, s0:s0 + sw],
                         start=False, stop=True)
        nc.vector.memset(kz0[HD:P, s0:s0 + sw], 0.0)
        nc.vector.memset(kz1[0:HD, s0:s0 + sw], 0.0)
        nc.vector.tensor_scalar_add(kz0[0:HD, s0:s0 + sw], psn[0:HD, :],
                                    bk_sb[0:HD, :])
        nc.vector.tensor_scalar_add(kz1[HD:P, s0:s0 + sw], psn[HD:P, :],
                                    bk_sb[HD:P, :])

    def q_chunk(s0, sw):
        psn = ps.tile([P, GRP * 512], F32, tag="sc", name="qps")[:, :sw]
        nc.tensor.matmul(psn, wq_sb[:, 0, :], x_lo[:, s0:s0 + sw],
                         start=True, stop=False)
        nc.tensor.matmul(psn, wq_sb[:, 1, :], x_hi[:, s0:s0 + sw],
                         start=False, stop=True)
        nc.vector.tensor_scalar_add(q_sb[:, s0:s0 + sw], psn, bq_sb)

    def vt_tiles3(tts):
        # one 3-bank PSUM tile for all three V t-tiles (1 pool slot, not 3)
        psn = ps.tile([P, GRP * 512], F32, tag="sc", name="vtps")
        for j, tt in enumerate(tts):
            sub = psn[:, j * P:(j + 1) * P]
            nc.tensor.matmul(sub, x_lo[:, tt * P:(tt + 1) * P], wv_sb[:, 0, :],
                             start=True, stop=False, skip_group_check=True)
            nc.tensor.matmul(sub, x_hi[:, tt * P:(tt + 1) * P], wv_sb[:, 1, :],
                             start=False, stop=True, skip_group_check=True)
            nc.vector.tensor_add(vt_sb[:, tt, 0:HD], sub[:, 0:HD], bv_bc[:, 0:HD])
            nc.vector.tensor_add(vt_sb[:, tt, 65:65 + HD], sub[:, HD:P],
                                 bv_bc[:, HD:P])

    def emit_av(pend):
        ex, g, ot, h, sw = pend
        for j in range(GRP):
            tt = g * GRP + j
            nc.tensor.matmul(ot, vt_sb[:, tt, h * 65:(h + 1) * 65],
                             ex[:, j * sw:(j + 1) * sw],
                             start=(tt == 0), stop=(tt == TT - 1))

    def wo_half(s0, sw, half):
        psn = ps.tile([P, GRP * 512], F32, tag="sc", name="wops")[:, :sw]
        nc.tensor.matmul(psn, wot_sb[:, half * P:(half + 1) * P],
                         attn_full[:, s0:s0 + sw], start=True, stop=True)
        ob = wo_out.tile([P, 512], F32, tag="ob", name="ob")[:, :sw]
        nc.vector.tensor_copy(ob, psn)
        # mid-kernel outputs ride the gpsimd queue (fire-and-forget), keeping
        # the sync queue short so the a1 partition-shift DMAs complete fast;
        # the last-scheduled chunk (3) goes on sync so the gpsimd queue is
        # drained well before the end-of-kernel ritual
        eng = nc.sync if s0 == 1536 else nc.gpsimd
        eng.dma_start(out=t_out[half * P:(half + 1) * P, s0:s0 + sw],
                      in_=ob)

    norm_ctr = [0]

    def emit_norm(ot, h, s0, sw, shift=True):
        norm_ctr[0] += 1
        comb = nrm.tile([65, 512], F32, tag="comb", name="comb")[:, :sw]
        nc.vector.tensor_copy(comb, ot)
        # broadcast the sums row (64) to all partitions via the ones64
        # stationary (full 128x128 PE config - no tile-config switch), then
        # reciprocal partition-parallel; all hops stay on-chip (no DMA).
        nc.vector.tensor_copy(comb16[HD:HD + 1, :sw], comb[HD:HD + 1, :])
        bc = ps.tile([P, 512], F32, tag="ot", name="bc")[:, :sw]
        nc.tensor.matmul(bc, ones64, comb16[:, :sw], start=True, stop=True)
        rb = nrm.tile([HD, 512], F32, tag="rb", name="rb")[:, :sw]
        nc.vector.reciprocal_approx_fast(rb, bc[0:HD, :])
        if h == 0:
            nc.vector.tensor_mul(attn_full[0:HD, s0:s0 + sw], comb[0:HD, :],
                                 rb)
            return None
        a1 = nrm.tile([HD, 512], BF16, tag="a1", name="a1")[:, :sw]
        nc.vector.tensor_mul(a1, comb[0:HD, :], rb)
        if shift:
            nc.sync.dma_start(out=attn_full[HD:P, s0:s0 + sw], in_=a1)
        return a1

    # ---- PE warm-up in the initial DMA-wait window: the p-state ramps with
    # continuous work, so the first real projections run at full clock ----
    warm = ps.tile([P, 512], F32, tag="ot", name="warm")
    for _ in range(12):
        nc.tensor.matmul(warm[:, 0:P], ones64, ones64,
                         start=True, stop=True, skip_group_check=True)

    # ---- minimal pre-work: K and Q of s-chunk 0, in 256-halves so the
    # first projection matmuls overlap the trailing x DMA ----
    k_chunk(0, 256)
    k_chunk(256, 256)
    q_chunk(0, 256)
    q_chunk(256, 256)

    # weave table: (unit, group) -> work emitted between scores and exp.
    # unit 0 carries the remaining K chunks + all V tiles (each AV batch g is
    # consumed at g+1, so VT 3g..3g+2 emitted at slot g are just in time);
    # units 1-2 carry the remaining Q chunks; Wo of s-chunk c runs in unit
    # 2c+2 (one full unit after its head-B normalization started).
    W = {}
    for g in range(6):
        W[(0, g)] = [lambda tts=tuple(range(3 * g, 3 * g + 3)): vt_tiles3(tts)]
    for i, (s0, sw) in enumerate(S_CHUNKS[1:]):
        W[(0, i)].append(lambda a=s0, b=sw: k_chunk(a, b))
    q_halves = [(512, 256), (768, 256), (1024, 256), (1280, 256),
                (1536, 256), (1792, 256), (2048, 256)]
    q_slots = [(1, 1), (1, 2), (1, 3), (1, 4), (2, 1), (2, 2), (3, 1)]
    for (u, g), (s0, sw) in zip(q_slots, q_halves):
        W.setdefault((u, g), []).append(lambda a=s0, b=sw: q_chunk(a, b))
    for c, (s0, sw) in enumerate(S_CHUNKS[:4]):
        g0 = 2 if c == 3 else 1  # chunk 3's unit-9 groups are short (sw=256)
        W.setdefault((2 * c + 3, g0), []).append(
            lambda a=s0, b=sw: wo_half(a, b, 0))
        W.setdefault((2 * c + 3, g0 + 1), []).append(
            lambda a=s0, b=sw: wo_half(a, b, 1))

    units = [(s0, sw, h) for (s0, sw) in S_CHUNKS for h in (0, 1)]
    pend_q = deque()   # (ex, g, ot, h, sw): exp batches awaiting attn@V
    norm_q = deque()   # (ot, h, s0, sw): units awaiting normalization
    for u, (s0, sw, h) in enumerate(units):
        kz = kz0 if h == 0 else kz1
        ot = ps.tile([P, 512], F32, tag="ot", name="ot")[0:65, :sw]
        for g in range(TT // GRP):
            sc = ps.tile([P, GRP * 512], F32, tag="sc", name="sc")[:, :GRP * sw]
            for j in range(GRP):
                tt = g * GRP + j
                nc.tensor.matmul(sc[:, j * sw:(j + 1) * sw],
                                 kz[:, tt * P:(tt + 1) * P],
                                 q_sb[:, s0:s0 + sw],
                                 start=True, stop=True)
            for thunk in W.get((u, g), []):
                thunk()
            if len(pend_q) >= 2:
                p0 = pend_q.popleft()
                emit_av(p0)
                if p0[1] == TT // GRP - 1:  # last batch of its unit
                    emit_norm(*norm_q.popleft())
            ex = ex_pool.tile([P, GRP * 512], BF16, tag="ex", name="ex")[:, :GRP * sw]
            nc.scalar.activation(ex, sc, mybir.ActivationFunctionType.Exp,
                                 scale=SCALE)
            pend_q.append((ex, g, ot, h, sw))
            if g == TT // GRP - 1:
                norm_q.append((ot, h, s0, sw))
    while len(pend_q) > 1:
        p0 = pend_q.popleft()
        emit_av(p0)
        if p0[1] == TT // GRP - 1:
            emit_norm(*norm_q.popleft())
    emit_av(pend_q.popleft())
    # tail: last unit is (s-chunk 4, head B).  Normalize without the
    # partition-shift DMA and fold the shift into a split Wo contraction.
    ls0, lsw = S_CHUNKS[-1]
    a1_last = emit_norm(*norm_q.popleft(), shift=False)
    for half in range(2):
        psn = ps.tile([P, GRP * 512], F32, tag="sc", name="wolast")[:, :lsw]
        nc.tensor.matmul(psn, wot_sb[0:HD, half * P:(half + 1) * P],
                         attn_full[0:HD, ls0:ls0 + lsw], start=True, stop=False)
        nc.tensor.matmul(psn, wot_hi[:, half * P:(half + 1) * P],
                         a1_last, start=False, stop=True)
        ob = wo_out.tile([P, 512], F32, tag="ob", name="ob")[:, :lsw]
        nc.vector.tensor_copy(ob, psn)
        eng = nc.sync if half == 0 else nc.gpsimd
        eng.dma_start(out=t_out[half * P:(half + 1) * P, ls0:ls0 + lsw],
                      in_=ob)

    wo_out.release()
    nrm.release()
    ex_pool.release()
    ps.release()
    singles.release()


_NC_CACHE = {}


def build_nc():
    if "nc" not in _NC_CACHE:
        nc = bacc.Bacc("TRN2", target_bir_lowering=False, debug=False, num_devices=8)
        with tile.TileContext(nc) as tc:
            _body(tc)
        nc.compile()
        _NC_CACHE["nc"] = nc
    return _NC_CACHE["nc"]


def make_in_maps(x, Wq, bq, Wk, bk, Wv, bv, Wo, bo):
    import ml_dtypes
    bf16 = ml_dtypes.bfloat16
    N = x.shape[0]
    xf = np.ascontiguousarray(np.asarray(x, np.float32).reshape(N, C, S).astype(bf16))
    in_maps = []
    for c in range(8):
        n, hp = c // 2, c % 2
        ch = slice(hp * P, (hp + 1) * P)
        wot = np.ascontiguousarray(np.asarray(Wo, np.float32)[:, ch].T.astype(bf16))  # (128, 256)
        in_maps.append({
            "x": xf[n],
            "wqt": np.ascontiguousarray(np.asarray(Wq, np.float32)[ch].T.astype(bf16)),
            "wkt": np.ascontiguousarray(np.asarray(Wk, np.float32)[ch].T.astype(bf16)),
            "wvt": np.ascontiguousarray(np.asarray(Wv, np.float32)[ch].T.astype(bf16)),
            "wot": wot,
            "bq": np.ascontiguousarray(np.asarray(bq, np.float32)[ch].reshape(P, 1)),
            "bk": np.ascontiguousarray(np.asarray(bk, np.float32)[ch].reshape(P, 1)),
            "bv": np.ascontiguousarray(np.asarray(bv, np.float32)[ch].reshape(1, P)),
        })
    return in_maps


def run(inputs, **kwargs):
    """Run on 8 cores; returns (full output, BassKernelResults)."""
    nc = build_nc()
    in_maps = make_in_maps(**inputs)
    res = run_bass_kernel_spmd(nc, in_maps, core_ids=list(range(8)), **kwargs)
    x = np.asarray(inputs["x"], np.float32)
    bo = np.asarray(inputs["bo"], np.float32)
    N, _, H, W = x.shape
    out = np.empty((N, C, S), np.float32)
    for n in range(N):
        out[n] = (x[n].reshape(C, S)
                  + res.results[2 * n]["out"]
                  + res.results[2 * n + 1]["out"]
                  + bo[:, None])
    return out.reshape(N, C, H, W), res


def kernel(**inputs):
    out, _ = run(inputs)
    return out
